# revision 39
# baseline (speedup 1.0000x reference)
"""Trainium2 Bass kernel for CompressedGlobalAttention (v2, bf16).

Problem (hardcoded from the reference):
  x: (2, 8192, 1024) fp32, local_window_start=4096, 16 heads x 64 dim,
  compression ratio 8 -> 512 avg-pooled KV "pools" from the first 4096
  tokens of each batch.  out = softmax(mask(q @ k_c^T)) @ v_c projected.

Sharding (8 cores): core = b*4 + qi handles batch b, query rows
[qi*2048, (qi+1)*2048).  Each core recomputes the pooled k/v for its
batch locally; outputs are disjoint row blocks -> no collectives.

v2 design vs v1:
  - every matmul operand is bf16 (fp32 PSUM accumulate); inputs are
    converted/transposed on the host, halving HBM traffic.
  - x arrives pre-transposed (xqT, xpT) so no PE transposes at all.
  - pooling = one DVE windowed-reduce per 128-row chunk of xpT
    (sum of 8 adjacent columns); the 1/8 is folded into Wk/Wv host-side.
  - causal mask is a 0/1 multiplicative bf16 mask applied to exp(scores)
    (identical to additive -inf pre-exp); exp runs as one wide
    [128, 2048] activation per head, no per-chunk bias plumbing.
  - softmax denominator comes from an appended ones-column of V
    (oa row 64), gathered per head via tiny SBUF DMAs.
  - rows 0..7 of each batch attend to nothing; the host overwrites them
    with the analytic uniform-attention value (reference behavior).
"""

import os
import sys

import numpy as np

NUM_HEADS = 16
HEAD_DIM = 64
RATIO = 8
B, S, D = 2, 8192, 1024
LWS = 4096
NPOOL = LWS // RATIO        # 512
SQ = S // 4                 # 2048 query rows per core
N_CORES = 8
ST = 512                    # seq tile in phase B
NST = SQ // ST              # 4 seq tiles per core

_RUNNER = None


def _ensure_path():
    for p in ("/opt/trn_rl_repo",):
        if p not in sys.path and os.path.isdir(p):
            sys.path.insert(0, p)


def xpt_dram_quarter(xpT, m, q):
    return xpT[m * 128 : (m + 1) * 128, q * 1024 : (q + 1) * 1024]


def build_program():
    """Build the Bass/Tile SPMD program (same for all 8 cores)."""
    _ensure_path()
    import concourse.bass as bass
    import concourse.mybir as mybir
    import concourse.tile as tile
    from contextlib import ExitStack

    f32 = mybir.dt.float32
    bf16 = mybir.dt.bfloat16
    Exp = mybir.ActivationFunctionType.Exp

    nc = bass.Bass("TRN2", target_bir_lowering=False, debug=False)

    xqT = nc.declare_dram_parameter("xqT", [D, SQ], bf16, isOutput=False)
    xpT = nc.declare_dram_parameter("xpT", [D, LWS], bf16, isOutput=False)
    wq = nc.declare_dram_parameter("wq", [D, D], bf16, isOutput=False)
    wk = nc.declare_dram_parameter("wk", [D, D], bf16, isOutput=False)
    wv = nc.declare_dram_parameter("wv", [D, D], bf16, isOutput=False)
    wo = nc.declare_dram_parameter("wo", [D, D], bf16, isOutput=False)
    bq2 = nc.declare_dram_parameter("bq2", [128, 8], f32, isOutput=False)
    bk2 = nc.declare_dram_parameter("bk2", [128, 8], f32, isOutput=False)
    bvr = nc.declare_dram_parameter("bvr", [1, D], bf16, isOutput=False)
    bor = nc.declare_dram_parameter("bor", [1, D], bf16, isOutput=False)
    hsd = nc.declare_dram_parameter("headsel", [16, D], bf16, isOutput=False)
    emd = nc.declare_dram_parameter("emask", [128, NST * 4 * ST], bf16, isOutput=False)
    yout = nc.declare_dram_parameter("y", [SQ, D], bf16, isOutput=True)

    with tile.TileContext(nc) as tc, ExitStack() as top:
        # ---------------- persistent pools ----------------
        consts = top.enter_context(tc.tile_pool(name="consts", bufs=1))
        kTp = top.enter_context(tc.tile_pool(name="kTp", bufs=1))
        vap = top.enter_context(tc.tile_pool(name="vap", bufs=1))

        headsel = consts.tile([16, D], bf16, name="headsel")
        nc.sync.dma_start(headsel[:], hsd[:, :])
        bq2_sb = consts.tile([128, 8], f32, name="bq2_sb")
        nc.sync.dma_start(bq2_sb[:], bq2[:, :])
        bor_sb = consts.tile([1, D], bf16, name="bor_sb")
        nc.sync.dma_start(bor_sb[:], bor[:, :])
        ones1 = consts.tile([1, 128], bf16, name="ones1")
        nc.vector.memset(ones1[:], 1.0)

        kT = [kTp.tile([128, NPOOL], bf16, name=f"kT{j}", tag=f"kT{j}") for j in range(8)]
        vaug = [
            vap.tile([128, NUM_HEADS * (HEAD_DIM + 1)], bf16, name=f"vaug{i}", tag=f"vaug{i}")
            for i in range(4)
        ]

        # phase-B weights + per-tile input prefetch (hoisted so the DMAs and
        # the first q-projection can overlap phase A)
        wqop = top.enter_context(tc.tile_pool(name="wqop", bufs=1))
        xqp = top.enter_context(tc.tile_pool(name="xqp", bufs=2))
        emp = top.enter_context(tc.tile_pool(name="emp", bufs=2))
        dnp = top.enter_context(tc.tile_pool(name="dnp", bufs=2))
        wq_sb = [wqop.tile([128, D], bf16, name=f"wq{m}", tag=f"wq{m}") for m in range(8)]
        wo_sb = [wqop.tile([128, D], bf16, name=f"wo{j}", tag=f"wo{j}") for j in range(8)]
        for m in range(8):
            nc.sync.dma_start(wq_sb[m][:], wq[m * 128 : (m + 1) * 128, :])

        def load_tile_inputs(st, dma=True):
            s0 = st * ST
            xqt = [
                xqp.tile([128, ST], bf16, name=f"xqt{m}", tag=f"xqt_t{st}_{m}", bufs=1)
                for m in range(8)
            ]
            if dma:
                for m in range(8):
                    nc.sync.dma_start(
                        xqt[m][:], xqT[m * 128 : (m + 1) * 128, s0 : s0 + ST]
                    )
            emask = emp.tile([128, 4 * ST], bf16, name="emask", tag=f"emask_t{st}", bufs=1)
            nc.sync.dma_start(emask[:], emd[:, st * 4 * ST : (st + 1) * 4 * ST])
            return xqt, emask

        qpad = consts.tile([1, 128], f32, name="qpad")
        qobs = consts.tile([1, 512], bf16, name="qobs")
        _obs = [0]

        def obs_slice():
            i = _obs[0] % 64
            _obs[0] += 1
            return qobs[:, i * 8 : i * 8 + 8]
        prefetched = {0: load_tile_inputs(0)}
        # tile-1 inputs stream in between the pooling chunks of phase A so
        # tile-1's q-projection can keep PE fed while pooling runs
        xqt1, emask1 = load_tile_inputs(1, dma=False)

        # ---------------- phase A: pooled k/v ----------------
        with ExitStack() as pa:
            aconsts = pa.enter_context(tc.tile_pool(name="aconsts", bufs=1))
            wkvp = pa.enter_context(tc.tile_pool(name="wkvp", bufs=1))
            xpp = pa.enter_context(tc.tile_pool(name="xpp", bufs=2))
            plfp = pa.enter_context(tc.tile_pool(name="plfp", bufs=2))
            pltp = pa.enter_context(tc.tile_pool(name="pltp", bufs=1))

            bk2_sb = aconsts.tile([128, 8], f32, name="bk2_sb")
            nc.sync.dma_start(bk2_sb[:], bk2[:, :])
            bvr_sb = aconsts.tile([1, D], bf16, name="bvr_sb")
            nc.sync.dma_start(bvr_sb[:], bvr[:, :])

            wk_sb = [wkvp.tile([128, D], bf16, name=f"wk{m}", tag=f"wk{m}") for m in range(8)]
            wv_sb = [wkvp.tile([128, D], bf16, name=f"wv{m}", tag=f"wv{m}") for m in range(8)]

            # pooledT[m][c, p] = sum of the 8 tokens of pool p (1/8 folded
            # into wk/wv host-side); wk/wv loads interleave with the
            # pooling stream so the kv matmuls can chase the pooled chunks
            pooledT = [pltp.tile([128, NPOOL], bf16, name=f"pooledT{m}", tag=f"pooledT{m}") for m in range(8)]
            for m in range(8):
                xpt = xpp.tile([128, LWS], bf16, name="xpt", tag="xpt", bufs=2)
                nc.gpsimd.dma_start(xpt[:], xpT[m * 128 : (m + 1) * 128, :])
                for w in range(3):
                    nc.gpsimd.dma_start(
                        qpad[:, (3 * m + w) % 8 * 16 : (3 * m + w) % 8 * 16 + 8],
                        bq2[0:1, 0:8],
                    )
                for q in range(4):
                    # pairwise 8->1 sum tree along the free axis (bf16)
                    xq_ = xpt[:, q * 1024 : (q + 1) * 1024]
                    t1 = plfp.tile([128, 512], bf16, name="t1", tag="t1", bufs=2)
                    nc.vector.tensor_add(
                        t1[:],
                        xq_.rearrange("p (n r) -> p n r", r=2)[:, :, 0],
                        xq_.rearrange("p (n r) -> p n r", r=2)[:, :, 1],
                    )
                    t2 = plfp.tile([128, 256], bf16, name="t2", tag="t2", bufs=2)
                    nc.vector.tensor_add(
                        t2[:],
                        t1[:].rearrange("p (n r) -> p n r", r=2)[:, :, 0],
                        t1[:].rearrange("p (n r) -> p n r", r=2)[:, :, 1],
                    )
                    nc.vector.tensor_add(
                        pooledT[m][:, q * 128 : (q + 1) * 128],
                        t2[:].rearrange("p (n r) -> p n r", r=2)[:, :, 0],
                        t2[:].rearrange("p (n r) -> p n r", r=2)[:, :, 1],
                    )
                nc.gpsimd.tensor_copy(obs_slice(), t1[0:1, 0:8])
                nc.sync.dma_start(
                    xqt1[m][:], xqT[m * 128 : (m + 1) * 128, ST : 2 * ST]
                )
                nc.sync.dma_start(wk_sb[m][:], wk[m * 128 : (m + 1) * 128, :])
                nc.sync.dma_start(wv_sb[m][:], wv[m * 128 : (m + 1) * 128, :])

            # kT[j][d, p] = sum_c Wk[c, d] pooledT[c, p] + bk[d]
            with tc.tile_pool(name="kv_ps", bufs=2, space="PSUM") as kv_ps:
                for j in range(8):
                    ps = kv_ps.tile([128, NPOOL], f32, name="ps2", tag="kvps")
                    for m in range(8):
                        nc.tensor.matmul(
                            ps[:],
                            wk_sb[m][:, j * 128 : (j + 1) * 128],
                            pooledT[m][:],
                            start=(m == 0),
                            stop=(m == 7),
                        )
                    nc.scalar.add(kT[j][:], ps[:], bk2_sb[:, j : j + 1])

            # v[p, d] = sum_c pooled[p, c] Wv[c, d] + bv[d]; augment ones col
            with tc.tile_pool(name="v_ps", bufs=2, space="PSUM") as v_ps:
                for i in range(4):
                    ps = v_ps.tile([128, D], f32, name="ps3", tag="vps")
                    for m in range(8):
                        for h2 in range(2):
                            nc.tensor.matmul(
                                ps[:, h2 * 512 : (h2 + 1) * 512],
                                pooledT[m][:, i * 128 : (i + 1) * 128],
                                wv_sb[m][:, h2 * 512 : (h2 + 1) * 512],
                                start=(m == 0),
                                stop=False,
                            )
                    for h2 in range(2):
                        nc.tensor.matmul(
                            ps[:, h2 * 512 : (h2 + 1) * 512],
                            ones1[:],
                            bvr_sb[:, h2 * 512 : (h2 + 1) * 512],
                            start=False,
                            stop=True,
                        )
                    va = vaug[i][:].rearrange("p (h x) -> p h x", x=HEAD_DIM + 1)
                    nc.scalar.copy(
                        va[:, :, 0:HEAD_DIM],
                        ps[:].rearrange("p (h x) -> p h x", x=HEAD_DIM),
                    )
                    nc.vector.memset(va[:, :, HEAD_DIM : HEAD_DIM + 1], 1.0)

        prefetched[1] = (xqt1, emask1)
        for j in range(8):
            nc.sync.dma_start(wo_sb[j][:], wo[j * 128 : (j + 1) * 128, :])
        # warm all 8 SWDGE queues so later data DMAs carry only
        # {RAW, self-queue} waits (codegen allows at most 2)
        qwarm = consts.tile([1, 128], f32, name="qwarm")
        for w in range(8):
            nc.gpsimd.dma_start(qwarm[:, w * 16 : w * 16 + 8], bq2[0:1, 0:8])

        # ---------------- phase B: attention ----------------
        with ExitStack() as pb:
            qTp = pb.enter_context(tc.tile_pool(name="qTp", bufs=2))
            ep = pb.enter_context(tc.tile_pool(name="ep", bufs=3))
            oTp = pb.enter_context(tc.tile_pool(name="oTp", bufs=2))
            ysp = pb.enter_context(tc.tile_pool(name="ysp", bufs=2))
            # PSUM: sc 4 banks + oa 1 + q 1 + y/rps 2 = 8 banks
            scp = pb.enter_context(tc.tile_pool(name="scp", bufs=1, space="PSUM"))
            oap = pb.enter_context(tc.tile_pool(name="oap", bufs=1, space="PSUM"))
            qpp = pb.enter_context(tc.tile_pool(name="qpp", bufs=1, space="PSUM"))
            ypp = pb.enter_context(tc.tile_pool(name="ypp", bufs=2, space="PSUM"))

            prev_recips = None
            for st in range(NST):
                s0 = st * ST
                xqt, emask = prefetched.pop(st)
                if st + 2 < NST and st + 2 not in prefetched:
                    prefetched[st + 2] = load_tile_inputs(st + 2)
                # q^T[d, s] for this tile
                qT = [qTp.tile([128, ST], bf16, name=f"qT{j}", tag=f"qT{j}") for j in range(8)]
                for j in range(8):
                    ps = qpp.tile([128, ST], f32, name="qps", tag="qps")
                    for m in range(8):
                        nc.tensor.matmul(
                            ps[:],
                            wq_sb[m][:, j * 128 : (j + 1) * 128],
                            xqt[m][:],
                            start=(m == 0),
                            stop=(m == 7),
                        )
                    nc.scalar.add(qT[j][:], ps[:], bq2_sb[:, j : j + 1])

                oT = [oTp.tile([128, ST], bf16, name=f"oT{j}", tag=f"oT{j}") for j in range(8)]
                # head h -> partition 32*(h%4), free slice (h//4)*ST; the
                # strided gather below lands head h at denoms row (h%4)*4+h//4
                # (headsel is permuted on the host to match)
                drows = dnp.tile([128, 4 * ST], f32, name="drows", tag="drows")
                for h in range(NUM_HEADS):
                    j, r0 = h // 2, 64 * (h % 2)
                    sc = scp.tile([128, 4 * ST], f32, name="sc", tag="sc")
                    for pc in range(4):
                        nc.tensor.matmul(
                            sc[:, pc * ST : (pc + 1) * ST],
                            kT[j][r0 : r0 + 64, pc * 128 : (pc + 1) * 128],
                            qT[j][r0 : r0 + 64, :],
                            start=True,
                            stop=True,
                        )
                    e = ep.tile([128, 4 * ST], bf16, name="e", tag="e")
                    nc.scalar.activation(
                        e[:], sc[:], Exp, scale=float(1.0 / np.sqrt(HEAD_DIM))
                    )
                    nc.vector.tensor_mul(e[:], e[:], emask[:])
                    oa = oap.tile([HEAD_DIM + 1, ST], f32, name="oa", tag="oa")
                    for pc in range(4):
                        nc.tensor.matmul(
                            oa[:],
                            vaug[pc][:, h * 65 : h * 65 + 65],
                            e[:, pc * ST : (pc + 1) * ST],
                            start=(pc == 0),
                            stop=(pc == 3),
                        )
                    nc.vector.tensor_copy(oT[j][r0 : r0 + 64, :], oa[0:HEAD_DIM, :])
                    nc.scalar.copy(
                        drows[32 * (h % 4) : 32 * (h % 4) + 1,
                              (h // 4) * ST : (h // 4 + 1) * ST],
                        oa[HEAD_DIM : HEAD_DIM + 1, :],
                    )

                if prev_recips is not None:
                    nc.gpsimd.tensor_copy(obs_slice(), prev_recips[0:1, 0:8])
                nc.gpsimd.tensor_copy(obs_slice(), drows[0:1, 0:8])
                denoms = dnp.tile([16, ST], f32, name="denoms", tag="denoms")
                nc.gpsimd.dma_start(
                    denoms[:],
                    drows[0:128:32, :].rearrange("p (g s) -> p g s", s=ST),
                )
                for w in range(3):
                    nc.gpsimd.dma_start(
                        qpad[:, (3 * st + w) % 8 * 16 + 8 : (3 * st + w) % 8 * 16 + 16],
                        bq2[0:1, 0:8],
                    )
                recips = dnp.tile([16, ST], bf16, name="recips", tag="recips")
                rtmp = dnp.tile([16, ST], f32, name="rtmp", tag="rtmp")
                nc.vector.tensor_scalar_max(rtmp[:], denoms[:], 1e-30)
                nc.vector.reciprocal(rtmp[:], rtmp[:])
                nc.vector.tensor_copy(recips[:], rtmp[:])
                prev_recips = recips
                for j in range(8):
                    rps = ypp.tile([128, ST], f32, name="rps", tag="yps")
                    nc.tensor.matmul(
                        rps[:],
                        headsel[:, j * 128 : (j + 1) * 128],
                        recips[:],
                        start=True,
                        stop=True,
                    )
                    nc.vector.tensor_mul(oT[j][:], oT[j][:], rps[:])

                # final projection y[s, :] = O^T.T Wo + bo
                for q4 in range(4):
                    ysb = ysp.tile([128, D], bf16, name="ysb", tag="ysb")
                    for hf in range(2):
                        yh = ypp.tile([128, 512], f32, name="yh", tag="yps")
                        for j in range(8):
                            nc.tensor.matmul(
                                yh[:],
                                oT[j][:, q4 * 128 : (q4 + 1) * 128],
                                wo_sb[j][:, hf * 512 : (hf + 1) * 512],
                                start=(j == 0),
                                stop=False,
                            )
                        nc.tensor.matmul(
                            yh[:],
                            ones1[:],
                            bor_sb[:, hf * 512 : (hf + 1) * 512],
                            start=False,
                            stop=True,
                        )
                        nc.vector.tensor_copy(ysb[:, hf * 512 : (hf + 1) * 512], yh[:])
                    nc.gpsimd.tensor_copy(obs_slice(), ysb[0:1, 0:8])
                    nc.gpsimd.dma_start(yout[s0 + q4 * 128 : s0 + q4 * 128 + 128, :], ysb[:])

    return nc


# ---------------------------------------------------------------------------
# host side
# ---------------------------------------------------------------------------

def _bf16(a):
    import ml_dtypes

    return np.ascontiguousarray(np.asarray(a).astype(ml_dtypes.bfloat16))


def _core_emask(qi):
    """0/1 visibility mask, e-layout [128, NST*4*ST] for quarter qi."""
    p = np.arange(128)[:, None]
    out = np.empty((128, NST * 4 * ST), np.float32)
    for st in range(NST):
        sg = qi * SQ + st * ST + np.arange(ST)[None, :]
        for pc in range(4):
            blk = (sg >= 8 * (128 * pc + p) + 8).astype(np.float32)
            out[:, (st * 4 + pc) * ST : (st * 4 + pc + 1) * ST] = blk
    return out


def _numpy_reference(x, lws, Wq, bq, Wk, bk, Wv, bv, Wo, bo):
    Bx, Sx, Dx = x.shape
    H, Hd, R = NUM_HEADS, HEAD_DIM, RATIO
    if lws <= R:
        return np.zeros_like(x)
    npool = lws // R
    trunc = npool * R
    comp = x[:, :trunc, :].reshape(Bx, npool, R, Dx).mean(axis=2)
    q = (x @ Wq + bq).reshape(Bx, Sx, H, Hd).transpose(0, 2, 1, 3)
    k = (comp @ Wk + bk).reshape(Bx, npool, H, Hd).transpose(0, 2, 1, 3)
    v = (comp @ Wv + bv).reshape(Bx, npool, H, Hd).transpose(0, 2, 1, 3)
    scores = np.einsum("bhqd,bhkd->bhqk", q, k) / np.sqrt(Hd)
    mask = np.arange(Sx)[:, None] >= (np.arange(npool) + 1) * R
    scores = np.where(mask[None, None], scores, -1e9)
    scores = scores - scores.max(axis=-1, keepdims=True)
    e = np.exp(scores)
    attn = e / e.sum(axis=-1, keepdims=True)
    out = np.einsum("bhqk,bhkd->bhqd", attn, v)
    out = out.transpose(0, 2, 1, 3).reshape(Bx, Sx, H * Hd)
    return (out @ Wo + bo).astype(np.float32)


def make_in_maps(x, Wq, bq, Wk, bk, Wv, bv, Wo, bo):
    x = np.asarray(x, np.float32)
    headsel = np.zeros((16, D), np.float32)
    for r in range(16):
        h = (r % 4) * 4 + r // 4
        headsel[r, h * 64 : (h + 1) * 64] = 1.0

    wq_b = _bf16(Wq)
    wk_b = _bf16(np.asarray(Wk, np.float32) / RATIO)
    wv_b = _bf16(np.asarray(Wv, np.float32) / RATIO)
    wo_b = _bf16(Wo)
    bq2 = np.ascontiguousarray(np.asarray(bq, np.float32).reshape(8, 128).T)
    bk2 = np.ascontiguousarray(np.asarray(bk, np.float32).reshape(8, 128).T)
    bvr = _bf16(np.asarray(bv, np.float32).reshape(1, D))
    bor = _bf16(np.asarray(bo, np.float32).reshape(1, D))
    hs_b = _bf16(headsel)

    xpT_b = [_bf16(x[b, :LWS, :].T) for b in range(B)]
    emasks = [_bf16(_core_emask(qi)) for qi in range(4)]

    in_maps = []
    for core in range(N_CORES):
        b, qi = core // 4, core % 4
        in_maps.append(
            {
                "xqT": _bf16(x[b, qi * SQ : (qi + 1) * SQ, :].T),
                "xpT": xpT_b[b],
                "wq": wq_b,
                "wk": wk_b,
                "wv": wv_b,
                "wo": wo_b,
                "bq2": bq2,
                "bk2": bk2,
                "bvr": bvr,
                "bor": bor,
                "headsel": hs_b,
                "emask": emasks[qi],
            }
        )
    return in_maps


def assemble_output(x, Wv, bv, Wo, bo, results):
    y = np.empty((B, S, D), np.float32)
    for core in range(N_CORES):
        b, qi = core // 4, core % 4
        y[b, qi * SQ : (qi + 1) * SQ, :] = np.asarray(results[core]["y"], np.float32)
    # rows 0..7: all pools masked -> reference uses uniform attention
    for b in range(B):
        vmean = x[b, :LWS, :].astype(np.float64).mean(axis=0).astype(np.float32)
        row = (vmean @ Wv + bv) @ Wo + bo
        y[b, 0:8, :] = row[None, :]
    return y


def core_ref_slice(ref, core):
    b, qi = core // 4, core % 4
    return ref[b, qi * SQ : (qi + 1) * SQ, :]


def kernel(**inputs):
    x = np.asarray(inputs["x"], np.float32)
    lws = int(np.asarray(inputs["local_window_start"]))
    Wq = np.asarray(inputs["Wq"], np.float32)
    bq = np.asarray(inputs["bq"], np.float32)
    Wk = np.asarray(inputs["Wk"], np.float32)
    bk = np.asarray(inputs["bk"], np.float32)
    Wv = np.asarray(inputs["Wv"], np.float32)
    bv = np.asarray(inputs["bv"], np.float32)
    Wo = np.asarray(inputs["Wo"], np.float32)
    bo = np.asarray(inputs["bo"], np.float32)

    if lws != LWS or x.shape != (B, S, D):
        return _numpy_reference(x, lws, Wq, bq, Wk, bk, Wv, bv, Wo, bo)

    try:
        _ensure_path()
        from concourse.bass_utils import run_bass_kernel_spmd

        global _RUNNER
        if _RUNNER is None:
            _RUNNER = build_program()
        nc = _RUNNER

        in_maps = make_in_maps(x, Wq, bq, Wk, bk, Wv, bv, Wo, bo)
        res = run_bass_kernel_spmd(nc, in_maps, list(range(N_CORES)))
        return assemble_output(x, Wv, bv, Wo, bo, res.results)
    except Exception as ex:  # device path unavailable -> correct host fallback
        sys.stderr.write(f"kernel: device path failed ({type(ex).__name__}: {ex}); "
                         "using host fallback\n")
        return _numpy_reference(x, lws, Wq, bq, Wk, bk, Wv, bv, Wo, bo)


if __name__ == "__main__":
    np.random.seed(0)
    xs = np.random.randn(B, S, D).astype(np.float32)
    sc = 1.0 / np.sqrt(D)
    args = dict(
        x=xs,
        local_window_start=LWS,
        Wq=np.random.randn(D, D).astype(np.float32) * sc,
        bq=np.zeros(D, np.float32),
        Wk=np.random.randn(D, D).astype(np.float32) * sc,
        bk=np.zeros(D, np.float32),
        Wv=np.random.randn(D, D).astype(np.float32) * sc,
        bv=np.zeros(D, np.float32),
        Wo=np.random.randn(D, D).astype(np.float32) * sc,
        bo=np.zeros(D, np.float32),
    )
    y = kernel(**args)
    ref = _numpy_reference(
        xs, LWS, args["Wq"], args["bq"], args["Wk"], args["bk"],
        args["Wv"], args["bv"], args["Wo"], args["bo"],
    )
    err = np.abs(y - ref)
    rel = err.max() / np.abs(ref).max()
    print("max abs err:", err.max(), "rel:", rel)


# revision 41
# speedup vs baseline: 11591.2224x; 11591.2224x over previous
"""Trainium2 Bass kernel for CompressedGlobalAttention (v2, bf16).

Problem (hardcoded from the reference):
  x: (2, 8192, 1024) fp32, local_window_start=4096, 16 heads x 64 dim,
  compression ratio 8 -> 512 avg-pooled KV "pools" from the first 4096
  tokens of each batch.  out = softmax(mask(q @ k_c^T)) @ v_c projected.

Sharding (8 cores): core = b*4 + qi handles batch b, query rows
[qi*2048, (qi+1)*2048).  Each core recomputes the pooled k/v for its
batch locally; outputs are disjoint row blocks -> no collectives.

v2 design vs v1:
  - every matmul operand is bf16 (fp32 PSUM accumulate); inputs are
    converted/transposed on the host, halving HBM traffic.
  - x arrives pre-transposed (xqT, xpT) so no PE transposes at all.
  - pooling = one DVE windowed-reduce per 128-row chunk of xpT
    (sum of 8 adjacent columns); the 1/8 is folded into Wk/Wv host-side.
  - causal mask is a 0/1 multiplicative bf16 mask applied to exp(scores)
    (identical to additive -inf pre-exp); exp runs as one wide
    [128, 2048] activation per head, no per-chunk bias plumbing.
  - softmax denominator comes from an appended ones-column of V
    (oa row 64), gathered per head via tiny SBUF DMAs.
  - rows 0..7 of each batch attend to nothing; the host overwrites them
    with the analytic uniform-attention value (reference behavior).
"""

import os
import sys

import numpy as np

NUM_HEADS = 16
HEAD_DIM = 64
RATIO = 8
B, S, D = 2, 8192, 1024
LWS = 4096
NPOOL = LWS // RATIO        # 512
SQ = S // 4                 # 2048 query rows per core
N_CORES = 8
ST = 512                    # seq tile in phase B
NST = SQ // ST              # 4 seq tiles per core

_RUNNER = None


def _ensure_path():
    for p in ("/opt/trn_rl_repo",):
        if p not in sys.path and os.path.isdir(p):
            sys.path.insert(0, p)


def xpt_dram_quarter(xpT, m, q):
    return xpT[m * 128 : (m + 1) * 128, q * 1024 : (q + 1) * 1024]


def build_program():
    """Build the Bass/Tile SPMD program (same for all 8 cores)."""
    _ensure_path()
    import concourse.bass as bass
    import concourse.bacc as bacc
    import concourse.mybir as mybir
    import concourse.tile as tile
    from contextlib import ExitStack

    f32 = mybir.dt.float32
    bf16 = mybir.dt.bfloat16
    Exp = mybir.ActivationFunctionType.Exp

    nc = bacc.Bacc("TRN2", target_bir_lowering=False, debug=False)

    xqT = nc.declare_dram_parameter("xqT", [D, SQ], bf16, isOutput=False)
    xpT = nc.declare_dram_parameter("xpT", [D, LWS], bf16, isOutput=False)
    wq = nc.declare_dram_parameter("wq", [D, D], bf16, isOutput=False)
    wk = nc.declare_dram_parameter("wk", [D, D], bf16, isOutput=False)
    wv = nc.declare_dram_parameter("wv", [D, D], bf16, isOutput=False)
    wo = nc.declare_dram_parameter("wo", [D, D], bf16, isOutput=False)
    bq2 = nc.declare_dram_parameter("bq2", [128, 8], f32, isOutput=False)
    bk2 = nc.declare_dram_parameter("bk2", [128, 8], f32, isOutput=False)
    bvr = nc.declare_dram_parameter("bvr", [1, D], bf16, isOutput=False)
    bor = nc.declare_dram_parameter("bor", [1, D], bf16, isOutput=False)
    hsd = nc.declare_dram_parameter("headsel", [16, D], bf16, isOutput=False)
    emd = nc.declare_dram_parameter("emask", [128, NST * 4 * ST], bf16, isOutput=False)
    yout = nc.declare_dram_parameter("y", [SQ, D], bf16, isOutput=True)

    with tile.TileContext(nc) as tc, ExitStack() as top:
        # ---------------- persistent pools ----------------
        consts = top.enter_context(tc.tile_pool(name="consts", bufs=1))
        kTp = top.enter_context(tc.tile_pool(name="kTp", bufs=1))
        vap = top.enter_context(tc.tile_pool(name="vap", bufs=1))

        headsel = consts.tile([16, D], bf16, name="headsel")
        nc.sync.dma_start(headsel[:], hsd[:, :])
        bq2_sb = consts.tile([128, 8], f32, name="bq2_sb")
        nc.sync.dma_start(bq2_sb[:], bq2[:, :])
        bor_sb = consts.tile([1, D], bf16, name="bor_sb")
        nc.sync.dma_start(bor_sb[:], bor[:, :])
        ones1 = consts.tile([1, 128], bf16, name="ones1")
        nc.vector.memset(ones1[:], 1.0)
        atouch = consts.tile([1, 8], f32, name="atouch")
        nc.scalar.copy(atouch[:, 0:1], bq2_sb[0:1, 0:1])

        kT = [kTp.tile([128, NPOOL], bf16, name=f"kT{j}", tag=f"kT{j}") for j in range(8)]
        vaug = [
            vap.tile([128, NUM_HEADS * (HEAD_DIM + 1)], bf16, name=f"vaug{i}", tag=f"vaug{i}")
            for i in range(4)
        ]

        # phase-B weights + per-tile input prefetch (hoisted so the DMAs and
        # the first q-projection can overlap phase A)
        wqop = top.enter_context(tc.tile_pool(name="wqop", bufs=1))
        xqp = top.enter_context(tc.tile_pool(name="xqp", bufs=2))
        emp = top.enter_context(tc.tile_pool(name="emp", bufs=2))
        dnp = top.enter_context(tc.tile_pool(name="dnp", bufs=2))
        wq_sb = [wqop.tile([128, D], bf16, name=f"wq{m}", tag=f"wq{m}") for m in range(8)]
        wo_sb = [wqop.tile([128, D], bf16, name=f"wo{j}", tag=f"wo{j}") for j in range(8)]
        for m in range(8):
            nc.sync.dma_start(wq_sb[m][:], wq[m * 128 : (m + 1) * 128, :])

        def load_tile_inputs(st, dma=True):
            s0 = st * ST
            xqt = [
                xqp.tile([128, ST], bf16, name=f"xqt{m}", tag=f"xqt_t{st}_{m}", bufs=1)
                for m in range(8)
            ]
            if dma:
                for m in range(8):
                    nc.sync.dma_start(
                        xqt[m][:], xqT[m * 128 : (m + 1) * 128, s0 : s0 + ST]
                    )
            emask = emp.tile([128, 4 * ST], bf16, name="emask", tag=f"emask_t{st}", bufs=1)
            nc.sync.dma_start(emask[:], emd[:, st * 4 * ST : (st + 1) * 4 * ST])
            return xqt, emask

        qpad = consts.tile([1, 128], f32, name="qpad")
        qobs = consts.tile([1, 512], bf16, name="qobs")
        _obs = [0]

        def obs_slice():
            i = _obs[0] % 64
            _obs[0] += 1
            return qobs[:, i * 8 : i * 8 + 8]
        prefetched = {0: load_tile_inputs(0)}
        # tile-1 inputs stream in between the pooling chunks of phase A so
        # tile-1's q-projection can keep PE fed while pooling runs
        xqt1, emask1 = load_tile_inputs(1, dma=False)

        # ---------------- phase A: pooled k/v ----------------
        with ExitStack() as pa:
            aconsts = pa.enter_context(tc.tile_pool(name="aconsts", bufs=1))
            wkvp = pa.enter_context(tc.tile_pool(name="wkvp", bufs=1))
            xpp = pa.enter_context(tc.tile_pool(name="xpp", bufs=2))
            plfp = pa.enter_context(tc.tile_pool(name="plfp", bufs=2))
            pltp = pa.enter_context(tc.tile_pool(name="pltp", bufs=1))

            bk2_sb = aconsts.tile([128, 8], f32, name="bk2_sb")
            nc.sync.dma_start(bk2_sb[:], bk2[:, :])
            nc.scalar.copy(atouch[:, 1:2], bk2_sb[0:1, 0:1])
            bvr_sb = aconsts.tile([1, D], bf16, name="bvr_sb")
            nc.sync.dma_start(bvr_sb[:], bvr[:, :])

            wk_sb = [wkvp.tile([128, D], bf16, name=f"wk{m}", tag=f"wk{m}") for m in range(8)]
            wv_sb = [wkvp.tile([128, D], bf16, name=f"wv{m}", tag=f"wv{m}") for m in range(8)]

            # pooledT[m][c, p] = sum of the 8 tokens of pool p (1/8 folded
            # into wk/wv host-side); wk/wv loads interleave with the
            # pooling stream so the kv matmuls can chase the pooled chunks
            pooledT = [pltp.tile([128, NPOOL], bf16, name=f"pooledT{m}", tag=f"pooledT{m}") for m in range(8)]
            for m in range(8):
                xpt = xpp.tile([128, LWS], bf16, name="xpt", tag="xpt", bufs=2)
                nc.gpsimd.dma_start(xpt[:], xpT[m * 128 : (m + 1) * 128, :])
                for w in range(3):
                    nc.gpsimd.dma_start(
                        qpad[:, (3 * m + w) % 8 * 16 : (3 * m + w) % 8 * 16 + 8],
                        bq2[0:1, 0:8],
                    )
                for q in range(4):
                    # pairwise 8->1 sum tree along the free axis (bf16)
                    xq_ = xpt[:, q * 1024 : (q + 1) * 1024]
                    t1 = plfp.tile([128, 512], bf16, name="t1", tag="t1", bufs=2)
                    nc.vector.tensor_add(
                        t1[:],
                        xq_.rearrange("p (n r) -> p n r", r=2)[:, :, 0],
                        xq_.rearrange("p (n r) -> p n r", r=2)[:, :, 1],
                    )
                    t2 = plfp.tile([128, 256], bf16, name="t2", tag="t2", bufs=2)
                    nc.vector.tensor_add(
                        t2[:],
                        t1[:].rearrange("p (n r) -> p n r", r=2)[:, :, 0],
                        t1[:].rearrange("p (n r) -> p n r", r=2)[:, :, 1],
                    )
                    nc.vector.tensor_add(
                        pooledT[m][:, q * 128 : (q + 1) * 128],
                        t2[:].rearrange("p (n r) -> p n r", r=2)[:, :, 0],
                        t2[:].rearrange("p (n r) -> p n r", r=2)[:, :, 1],
                    )
                nc.gpsimd.tensor_copy(obs_slice(), t1[0:1, 0:8])
                nc.sync.dma_start(
                    xqt1[m][:], xqT[m * 128 : (m + 1) * 128, ST : 2 * ST]
                )
                nc.sync.dma_start(wk_sb[m][:], wk[m * 128 : (m + 1) * 128, :])
                nc.sync.dma_start(wv_sb[m][:], wv[m * 128 : (m + 1) * 128, :])

            # kT[j][d, p] = sum_c Wk[c, d] pooledT[c, p] + bk[d]
            with tc.tile_pool(name="kv_ps", bufs=2, space="PSUM") as kv_ps:
                for j in range(8):
                    ps = kv_ps.tile([128, NPOOL], f32, name="ps2", tag="kvps")
                    for m in range(8):
                        nc.tensor.matmul(
                            ps[:],
                            wk_sb[m][:, j * 128 : (j + 1) * 128],
                            pooledT[m][:],
                            start=(m == 0),
                            stop=(m == 7),
                        )
                    nc.scalar.add(kT[j][:], ps[:], bk2_sb[:, j : j + 1])

            # v[p, d] = sum_c pooled[p, c] Wv[c, d] + bv[d]; augment ones col
            with tc.tile_pool(name="v_ps", bufs=2, space="PSUM") as v_ps:
                for i in range(4):
                    ps = v_ps.tile([128, D], f32, name="ps3", tag="vps")
                    for m in range(8):
                        for h2 in range(2):
                            nc.tensor.matmul(
                                ps[:, h2 * 512 : (h2 + 1) * 512],
                                pooledT[m][:, i * 128 : (i + 1) * 128],
                                wv_sb[m][:, h2 * 512 : (h2 + 1) * 512],
                                start=(m == 0),
                                stop=False,
                            )
                    for h2 in range(2):
                        nc.tensor.matmul(
                            ps[:, h2 * 512 : (h2 + 1) * 512],
                            ones1[:],
                            bvr_sb[:, h2 * 512 : (h2 + 1) * 512],
                            start=False,
                            stop=True,
                        )
                    va = vaug[i][:].rearrange("p (h x) -> p h x", x=HEAD_DIM + 1)
                    nc.scalar.copy(
                        va[:, :, 0:HEAD_DIM],
                        ps[:].rearrange("p (h x) -> p h x", x=HEAD_DIM),
                    )
                    nc.vector.memset(va[:, :, HEAD_DIM : HEAD_DIM + 1], 1.0)

        prefetched[1] = (xqt1, emask1)
        for j in range(8):
            nc.sync.dma_start(wo_sb[j][:], wo[j * 128 : (j + 1) * 128, :])
        # warm all 8 SWDGE queues so later data DMAs carry only
        # {RAW, self-queue} waits (codegen allows at most 2)
        qwarm = consts.tile([1, 128], f32, name="qwarm")
        for w in range(8):
            nc.gpsimd.dma_start(qwarm[:, w * 16 : w * 16 + 8], bq2[0:1, 0:8])

        # ---------------- phase B: attention ----------------
        with ExitStack() as pb:
            qTp = pb.enter_context(tc.tile_pool(name="qTp", bufs=2))
            ep = pb.enter_context(tc.tile_pool(name="ep", bufs=3))
            oTp = pb.enter_context(tc.tile_pool(name="oTp", bufs=2))
            ysp = pb.enter_context(tc.tile_pool(name="ysp", bufs=2))
            # PSUM: sc 4 banks + oa 1 + q 1 + y/rps 2 = 8 banks
            scp = pb.enter_context(tc.tile_pool(name="scp", bufs=1, space="PSUM"))
            oap = pb.enter_context(tc.tile_pool(name="oap", bufs=1, space="PSUM"))
            qpp = pb.enter_context(tc.tile_pool(name="qpp", bufs=1, space="PSUM"))
            ypp = pb.enter_context(tc.tile_pool(name="ypp", bufs=2, space="PSUM"))

            prev_recips = None
            for st in range(NST):
                s0 = st * ST
                xqt, emask = prefetched.pop(st)
                if st + 2 < NST and st + 2 not in prefetched:
                    prefetched[st + 2] = load_tile_inputs(st + 2)
                # q^T[d, s] for this tile
                qT = [qTp.tile([128, ST], bf16, name=f"qT{j}", tag=f"qT{j}") for j in range(8)]
                for j in range(8):
                    ps = qpp.tile([128, ST], f32, name="qps", tag="qps")
                    for m in range(8):
                        nc.tensor.matmul(
                            ps[:],
                            wq_sb[m][:, j * 128 : (j + 1) * 128],
                            xqt[m][:],
                            start=(m == 0),
                            stop=(m == 7),
                        )
                    nc.scalar.add(qT[j][:], ps[:], bq2_sb[:, j : j + 1])

                oT = [oTp.tile([128, ST], bf16, name=f"oT{j}", tag=f"oT{j}") for j in range(8)]
                # head h -> partition 32*(h%4), free slice (h//4)*ST; the
                # strided gather below lands head h at denoms row (h%4)*4+h//4
                # (headsel is permuted on the host to match)
                drows = dnp.tile([128, 4 * ST], f32, name="drows", tag="drows")
                for h in range(NUM_HEADS):
                    j, r0 = h // 2, 64 * (h % 2)
                    sc = scp.tile([128, 4 * ST], f32, name="sc", tag="sc")
                    for pc in range(4):
                        nc.tensor.matmul(
                            sc[:, pc * ST : (pc + 1) * ST],
                            kT[j][r0 : r0 + 64, pc * 128 : (pc + 1) * 128],
                            qT[j][r0 : r0 + 64, :],
                            start=True,
                            stop=True,
                        )
                    e = ep.tile([128, 4 * ST], bf16, name="e", tag="e")
                    nc.scalar.activation(
                        e[:], sc[:], Exp, scale=float(1.0 / np.sqrt(HEAD_DIM))
                    )
                    nc.vector.tensor_mul(e[:], e[:], emask[:])
                    oa = oap.tile([HEAD_DIM + 1, ST], f32, name="oa", tag="oa")
                    for pc in range(4):
                        nc.tensor.matmul(
                            oa[:],
                            vaug[pc][:, h * 65 : h * 65 + 65],
                            e[:, pc * ST : (pc + 1) * ST],
                            start=(pc == 0),
                            stop=(pc == 3),
                        )
                    nc.vector.tensor_copy(oT[j][r0 : r0 + 64, :], oa[0:HEAD_DIM, :])
                    nc.scalar.copy(
                        drows[32 * (h % 4) : 32 * (h % 4) + 1,
                              (h // 4) * ST : (h // 4 + 1) * ST],
                        oa[HEAD_DIM : HEAD_DIM + 1, :],
                    )

                if prev_recips is not None:
                    nc.gpsimd.tensor_copy(obs_slice(), prev_recips[0:1, 0:8])
                nc.gpsimd.tensor_copy(obs_slice(), drows[0:1, 0:8])
                denoms = dnp.tile([16, ST], f32, name="denoms", tag="denoms")
                nc.gpsimd.dma_start(
                    denoms[:],
                    drows[0:128:32, :].rearrange("p (g s) -> p g s", s=ST),
                )
                for w in range(3):
                    nc.gpsimd.dma_start(
                        qpad[:, (3 * st + w) % 8 * 16 + 8 : (3 * st + w) % 8 * 16 + 16],
                        bq2[0:1, 0:8],
                    )
                recips = dnp.tile([16, ST], bf16, name="recips", tag="recips")
                rtmp = dnp.tile([16, ST], f32, name="rtmp", tag="rtmp")
                nc.vector.tensor_scalar_max(rtmp[:], denoms[:], 1e-30)
                nc.vector.reciprocal(rtmp[:], rtmp[:])
                nc.vector.tensor_copy(recips[:], rtmp[:])
                prev_recips = recips
                for j in range(8):
                    rps = ypp.tile([128, ST], f32, name="rps", tag="yps")
                    nc.tensor.matmul(
                        rps[:],
                        headsel[:, j * 128 : (j + 1) * 128],
                        recips[:],
                        start=True,
                        stop=True,
                    )
                    nc.vector.tensor_mul(oT[j][:], oT[j][:], rps[:])

                # final projection y[s, :] = O^T.T Wo + bo
                for q4 in range(4):
                    ysb = ysp.tile([128, D], bf16, name="ysb", tag="ysb")
                    for hf in range(2):
                        yh = ypp.tile([128, 512], f32, name="yh", tag="yps")
                        for j in range(8):
                            nc.tensor.matmul(
                                yh[:],
                                oT[j][:, q4 * 128 : (q4 + 1) * 128],
                                wo_sb[j][:, hf * 512 : (hf + 1) * 512],
                                start=(j == 0),
                                stop=False,
                            )
                        nc.tensor.matmul(
                            yh[:],
                            ones1[:],
                            bor_sb[:, hf * 512 : (hf + 1) * 512],
                            start=False,
                            stop=True,
                        )
                        nc.vector.tensor_copy(ysb[:, hf * 512 : (hf + 1) * 512], yh[:])
                    nc.gpsimd.tensor_copy(obs_slice(), ysb[0:1, 0:8])
                    nc.gpsimd.dma_start(yout[s0 + q4 * 128 : s0 + q4 * 128 + 128, :], ysb[:])

    nc.compile()
    return nc


# ---------------------------------------------------------------------------
# host side
# ---------------------------------------------------------------------------

def _bf16(a):
    import ml_dtypes

    return np.ascontiguousarray(np.asarray(a).astype(ml_dtypes.bfloat16))


def _core_emask(qi):
    """0/1 visibility mask, e-layout [128, NST*4*ST] for quarter qi."""
    p = np.arange(128)[:, None]
    out = np.empty((128, NST * 4 * ST), np.float32)
    for st in range(NST):
        sg = qi * SQ + st * ST + np.arange(ST)[None, :]
        for pc in range(4):
            blk = (sg >= 8 * (128 * pc + p) + 8).astype(np.float32)
            out[:, (st * 4 + pc) * ST : (st * 4 + pc + 1) * ST] = blk
    return out


def _numpy_reference(x, lws, Wq, bq, Wk, bk, Wv, bv, Wo, bo):
    Bx, Sx, Dx = x.shape
    H, Hd, R = NUM_HEADS, HEAD_DIM, RATIO
    if lws <= R:
        return np.zeros_like(x)
    npool = lws // R
    trunc = npool * R
    comp = x[:, :trunc, :].reshape(Bx, npool, R, Dx).mean(axis=2)
    q = (x @ Wq + bq).reshape(Bx, Sx, H, Hd).transpose(0, 2, 1, 3)
    k = (comp @ Wk + bk).reshape(Bx, npool, H, Hd).transpose(0, 2, 1, 3)
    v = (comp @ Wv + bv).reshape(Bx, npool, H, Hd).transpose(0, 2, 1, 3)
    scores = np.einsum("bhqd,bhkd->bhqk", q, k) / np.sqrt(Hd)
    mask = np.arange(Sx)[:, None] >= (np.arange(npool) + 1) * R
    scores = np.where(mask[None, None], scores, -1e9)
    scores = scores - scores.max(axis=-1, keepdims=True)
    e = np.exp(scores)
    attn = e / e.sum(axis=-1, keepdims=True)
    out = np.einsum("bhqk,bhkd->bhqd", attn, v)
    out = out.transpose(0, 2, 1, 3).reshape(Bx, Sx, H * Hd)
    return (out @ Wo + bo).astype(np.float32)


def make_in_maps(x, Wq, bq, Wk, bk, Wv, bv, Wo, bo):
    x = np.asarray(x, np.float32)
    headsel = np.zeros((16, D), np.float32)
    for r in range(16):
        h = (r % 4) * 4 + r // 4
        headsel[r, h * 64 : (h + 1) * 64] = 1.0

    wq_b = _bf16(Wq)
    wk_b = _bf16(np.asarray(Wk, np.float32) / RATIO)
    wv_b = _bf16(np.asarray(Wv, np.float32) / RATIO)
    wo_b = _bf16(Wo)
    bq2 = np.ascontiguousarray(np.asarray(bq, np.float32).reshape(8, 128).T)
    bk2 = np.ascontiguousarray(np.asarray(bk, np.float32).reshape(8, 128).T)
    bvr = _bf16(np.asarray(bv, np.float32).reshape(1, D))
    bor = _bf16(np.asarray(bo, np.float32).reshape(1, D))
    hs_b = _bf16(headsel)

    xpT_b = [_bf16(x[b, :LWS, :].T) for b in range(B)]
    emasks = [_bf16(_core_emask(qi)) for qi in range(4)]

    in_maps = []
    for core in range(N_CORES):
        b, qi = core // 4, core % 4
        in_maps.append(
            {
                "xqT": _bf16(x[b, qi * SQ : (qi + 1) * SQ, :].T),
                "xpT": xpT_b[b],
                "wq": wq_b,
                "wk": wk_b,
                "wv": wv_b,
                "wo": wo_b,
                "bq2": bq2,
                "bk2": bk2,
                "bvr": bvr,
                "bor": bor,
                "headsel": hs_b,
                "emask": emasks[qi],
            }
        )
    return in_maps


def assemble_output(x, Wv, bv, Wo, bo, results):
    y = np.empty((B, S, D), np.float32)
    for core in range(N_CORES):
        b, qi = core // 4, core % 4
        y[b, qi * SQ : (qi + 1) * SQ, :] = np.asarray(results[core]["y"], np.float32)
    # rows 0..7: all pools masked -> reference uses uniform attention
    for b in range(B):
        vmean = x[b, :LWS, :].astype(np.float64).mean(axis=0).astype(np.float32)
        row = (vmean @ Wv + bv) @ Wo + bo
        y[b, 0:8, :] = row[None, :]
    return y


def core_ref_slice(ref, core):
    b, qi = core // 4, core % 4
    return ref[b, qi * SQ : (qi + 1) * SQ, :]


def kernel(**inputs):
    x = np.asarray(inputs["x"], np.float32)
    lws = int(np.asarray(inputs["local_window_start"]))
    Wq = np.asarray(inputs["Wq"], np.float32)
    bq = np.asarray(inputs["bq"], np.float32)
    Wk = np.asarray(inputs["Wk"], np.float32)
    bk = np.asarray(inputs["bk"], np.float32)
    Wv = np.asarray(inputs["Wv"], np.float32)
    bv = np.asarray(inputs["bv"], np.float32)
    Wo = np.asarray(inputs["Wo"], np.float32)
    bo = np.asarray(inputs["bo"], np.float32)

    if lws != LWS or x.shape != (B, S, D):
        return _numpy_reference(x, lws, Wq, bq, Wk, bk, Wv, bv, Wo, bo)

    try:
        _ensure_path()
        from concourse.bass_utils import run_bass_kernel_spmd

        global _RUNNER
        if _RUNNER is None:
            _RUNNER = build_program()
        nc = _RUNNER

        in_maps = make_in_maps(x, Wq, bq, Wk, bk, Wv, bv, Wo, bo)
        res = run_bass_kernel_spmd(nc, in_maps, list(range(N_CORES)))
        return assemble_output(x, Wv, bv, Wo, bo, res.results)
    except Exception as ex:  # device path unavailable -> correct host fallback
        sys.stderr.write(f"kernel: device path failed ({type(ex).__name__}: {ex}); "
                         "using host fallback\n")
        return _numpy_reference(x, lws, Wq, bq, Wk, bk, Wv, bv, Wo, bo)


if __name__ == "__main__":
    np.random.seed(0)
    xs = np.random.randn(B, S, D).astype(np.float32)
    sc = 1.0 / np.sqrt(D)
    args = dict(
        x=xs,
        local_window_start=LWS,
        Wq=np.random.randn(D, D).astype(np.float32) * sc,
        bq=np.zeros(D, np.float32),
        Wk=np.random.randn(D, D).astype(np.float32) * sc,
        bk=np.zeros(D, np.float32),
        Wv=np.random.randn(D, D).astype(np.float32) * sc,
        bv=np.zeros(D, np.float32),
        Wo=np.random.randn(D, D).astype(np.float32) * sc,
        bo=np.zeros(D, np.float32),
    )
    y = kernel(**args)
    ref = _numpy_reference(
        xs, LWS, args["Wq"], args["bq"], args["Wk"], args["bk"],
        args["Wv"], args["bv"], args["Wo"], args["bo"],
    )
    err = np.abs(y - ref)
    rel = err.max() / np.abs(ref).max()
    print("max abs err:", err.max(), "rel:", rel)
